# revision 19
# baseline (speedup 1.0000x reference)
"""Trainium2 Bass kernel for the LSTM autoencoder (nn_LSTMAE).

Sharding: data-parallel over batch. B=1024 -> 128 rows per core on 8 cores.
Within a core everything lives in [feature, batch] layout (partition dim =
feature) so the recurrence needs no transposes; the input X is transposed
once up front with PE-transposes.

Math tricks (all weight prep host-side in numpy):
  - tanh(x) = 2*sigmoid(2x) - 1, and h is stored as h/2 with every
    h-consuming weight doubled.  Each gate then goes through one sigmoid
    activation (with its per-partition bias fused in) and the cell update
    is 4 fused scalar_tensor_tensor DVE ops:
        t' = (s_g - 0.5) * s_i          # = i * tanh(g) / 2
        q  = s_f * c
        c' = 2*t' + q                    # = f*c + i*tanh(g)
        s_c = sigmoid(2*c')
        h' = (s_c - 0.5) * s_o           # = h/2
  - decoder input projection folded into the recurrent weight:
        gates = (W_ih_dec @ W_out + W_hh_dec) h + (W_ih_dec b_out + biases)
  - per-gate PSUM banks + per-gate sigmoid so each sigma starts as soon as
    its own gate's matmul lands (gate order g,i,f,o = the order the DVE
    chain consumes them).
  - rec_err = sum_t,f |out^2 - x^2| accumulated every G steps with a fused
    abs-reduce; out[:, -1, :] is the first decoder projection.
"""

import numpy as np

B, T, F, H = 1024, 512, 64, 128
NCORES = 8
P = B // NCORES  # 128 batch rows per core
G4 = 4 * H  # 512
GERR = 8  # decoder error-accumulation block

_GSLICE = slice(2 * H, 3 * H)  # g-gate rows within the 4H axis (i,f,g,o)
# gate emission order: g first (t' needs it), then i, f, o
_GORDER = (2, 0, 1, 3)  # indices into (i, f, g, o)

# Two half-batch chains pipelined. Looks near-neutral in the cost model but
# measured ~70% SLOWER on hardware (the 2x instruction/semaphore traffic
# costs far more in reality than modeled), so it stays off.
import os as _os

SPLIT2 = _os.environ.get("LSTMAE_SPLIT2", "0") == "1"


def _prep_consts(inp):
    """Host-side weight preparation. Returns dict of numpy arrays shared by
    all cores."""
    f8 = np.float64

    W_ih_enc = np.asarray(inp["W_ih_enc"], f8)
    W_hh_enc = np.asarray(inp["W_hh_enc"], f8)
    b_enc = np.asarray(inp["b_ih_enc"], f8) + np.asarray(inp["b_hh_enc"], f8)
    W_ih_dec = np.asarray(inp["W_ih_dec"], f8)
    W_hh_dec = np.asarray(inp["W_hh_dec"], f8)
    b_dec0 = np.asarray(inp["b_ih_dec"], f8) + np.asarray(inp["b_hh_dec"], f8)
    W_out = np.asarray(inp["W_out"], f8)
    b_out = np.asarray(inp["b_out"], f8)

    # encoder: x-weights get g-rows doubled; h-weights doubled (+ g-rows)
    Wx_e = W_ih_enc.copy()
    Wh_e = 2.0 * W_hh_enc
    be = b_enc.copy()
    Wx_e[_GSLICE] *= 2.0
    Wh_e[_GSLICE] *= 2.0
    be[_GSLICE] *= 2.0

    # decoder: fold out-projection into recurrent weight; it multiplies h
    W_comb = 2.0 * (W_ih_dec @ W_out + W_hh_dec)
    bd = W_ih_dec @ b_out + b_dec0
    W_comb[_GSLICE] *= 2.0
    bd[_GSLICE] *= 2.0

    W_out_eff = 2.0 * W_out  # multiplies stored h/2

    f4 = np.float32
    c = {}
    wihT = np.ascontiguousarray(Wx_e.T, dtype=f4)  # [F, 4H]
    c["wihT"] = np.concatenate([wihT, wihT], axis=0)  # [128, 4H] parity copies
    c["whhT"] = np.ascontiguousarray(Wh_e.T, dtype=f4)  # [H, 4H]
    c["wcombT"] = np.ascontiguousarray(W_comb.T, dtype=f4)  # [H, 4H]
    c["benc"] = np.ascontiguousarray(be.reshape(4, H).T, dtype=f4)  # [H, 4]
    c["bdec"] = np.ascontiguousarray(bd.reshape(4, H).T, dtype=f4)  # [H, 4]
    # gate-pair bias rows for the split-2 path: pairs (g,i) and (f,o)
    be_r = be.reshape(4, H)
    bd_r = bd.reshape(4, H)
    c["bgi_enc"] = np.ascontiguousarray(be_r[[2, 0]], dtype=f4)  # [2, H]
    c["bfo_enc"] = np.ascontiguousarray(be_r[[1, 3]], dtype=f4)
    c["bgi_dec"] = np.ascontiguousarray(bd_r[[2, 0]], dtype=f4)
    c["bfo_dec"] = np.ascontiguousarray(bd_r[[1, 3]], dtype=f4)
    c["blk2"] = np.kron(np.eye(2, dtype=f4), np.ones((1, P // 2), f4))  # [2, P]
    c["ones1h"] = np.ones((1, P // 2), f4)
    c["woutT"] = np.ascontiguousarray(W_out_eff.T, dtype=f4)  # [H, F]
    c["bout"] = np.ascontiguousarray(b_out.reshape(1, F), dtype=f4)
    c["ones1"] = np.ones((1, P), f4)
    c["ident"] = np.eye(128, dtype=f4)
    return c


def _build_program(Tn):
    """Build the Bass/Tile program for a sequence length of Tn steps."""
    from contextlib import ExitStack

    import concourse.bacc as bacc
    import concourse.mybir as mybir
    import concourse.tile as tile

    dt = mybir.dt
    AF = mybir.ActivationFunctionType
    ALU = mybir.AluOpType
    AX = mybir.AxisListType
    f32 = dt.float32

    nc = bacc.Bacc("TRN2", target_bir_lowering=False, debug=False)

    x_d = nc.dram_tensor("x", [P, Tn, F], f32, kind="ExternalInput")
    cst_d = {
        name: nc.dram_tensor(name, list(shape), f32, kind="ExternalInput")
        for name, shape in [
            ("wihT", (128, G4)),
            ("whhT", (H, G4)),
            ("wcombT", (H, G4)),
            ("benc", (H, 4)),
            ("bdec", (H, 4)),
            ("bgi_enc", (2, H)),
            ("bfo_enc", (2, H)),
            ("bgi_dec", (2, H)),
            ("bfo_dec", (2, H)),
            ("blk2", (2, P)),
            ("ones1h", (1, P // 2)),
            ("woutT", (H, F)),
            ("bout", (1, F)),
            ("ones1", (1, P)),
            ("ident", (128, 128)),
        ]
    }
    cencT_d = nc.dram_tensor("c_encT", [H, P], f32, kind="ExternalOutput")
    recerr_d = nc.dram_tensor("rec_err", [P, 1], f32, kind="ExternalOutput")
    outlast_d = nc.dram_tensor("out_last", [P, F], f32, kind="ExternalOutput")

    n_blk = Tn // GERR

    with ExitStack() as ctx:
        tc = ctx.enter_context(tile.TileContext(nc))
        const = ctx.enter_context(tc.tile_pool(name="const", bufs=1))
        h_pool = ctx.enter_context(tc.tile_pool(name="hp", bufs=3))
        c_pool = ctx.enter_context(tc.tile_pool(name="cp", bufs=2))
        sg_pool = ctx.enter_context(tc.tile_pool(name="sg", bufs=2))
        si_pool = ctx.enter_context(tc.tile_pool(name="si", bufs=2))
        sf_pool = ctx.enter_context(tc.tile_pool(name="sf", bufs=2))
        so_pool = ctx.enter_context(tc.tile_pool(name="so", bufs=2))
        tmp_pool = ctx.enter_context(tc.tile_pool(name="tmp", bufs=3))
        big_pool = ctx.enter_context(tc.tile_pool(name="big", bufs=2))
        xT_pool = ctx.enter_context(tc.tile_pool(name="xT", bufs=1))
        xraw_pool = ctx.enter_context(tc.tile_pool(name="xraw", bufs=3))
        ubuf_pool = ctx.enter_context(tc.tile_pool(name="ubuf", bufs=2))
        err_pool = ctx.enter_context(tc.tile_pool(name="err", bufs=1))
        # PSUM budget (8 banks, bufs is per-tag): 4 gate banks, 2 transpose
        # banks (phase 0), 1-2 decoder out-projection banks.
        psum_g = ctx.enter_context(tc.tile_pool(name="psg", bufs=1, space="PSUM"))
        psum_tr = ctx.enter_context(tc.tile_pool(name="pstr", bufs=2, space="PSUM"))
        psum_o = ctx.enter_context(tc.tile_pool(name="pso", bufs=1, space="PSUM"))

        # ---- constants to SBUF
        cst = {}
        for name, d in cst_d.items():
            s = const.tile(list(d.shape), f32, tag=name)
            nc.sync.dma_start(s[:], d[:])
            cst[name] = s

        # ---- phase 0: transpose X into parity-packed [128, pair*P] layout
        # xT[:, pair*128:+128]: rows 0:64 = x_T(2*pair), rows 64:128 = x_T(2*pair+1)
        xT = xT_pool.tile([128, (Tn // 2) * P], f32)
        TC = 16  # timesteps per transpose chunk
        for c0 in range(0, Tn, TC):
            xr = xraw_pool.tile([128, TC * F], f32, tag="xr")
            nc.sync.dma_start(xr[:], x_d[:, c0 : c0 + TC, :])
            for pp in range(TC // 2):
                pt = psum_tr.tile([128, 128], f32, tag="ps_tr")
                nc.tensor.transpose(
                    pt[:], xr[:, (2 * pp) * F : (2 * pp + 2) * F], cst["ident"][:]
                )
                pair = c0 // 2 + pp
                dst = xT[:, pair * P : (pair + 1) * P]
                if pp % 2 == 0:
                    nc.scalar.copy(dst, pt[:])
                else:
                    nc.vector.tensor_copy(dst, pt[:])

        # ---- the shared LSTM cell tail (after per-gate sigmas are issued)
        def cell_tail(s_g, s_i, s_f, s_o, c_prev):
            tp = tmp_pool.tile([H, P], f32, tag="tp")
            nc.vector.scalar_tensor_tensor(
                tp[:], s_g[:], 0.5, s_i[:], op0=ALU.subtract, op1=ALU.mult
            )
            q = tmp_pool.tile([H, P], f32, tag="q")
            nc.vector.scalar_tensor_tensor(
                q[:], s_f[:], 1.0, c_prev[:], op0=ALU.mult, op1=ALU.mult
            )
            c_new = c_pool.tile([H, P], f32)
            nc.vector.scalar_tensor_tensor(
                c_new[:], tp[:], 2.0, q[:], op0=ALU.mult, op1=ALU.add
            )
            sc = tmp_pool.tile([H, P], f32, tag="sc")
            nc.scalar.activation(sc[:], c_new[:], AF.Sigmoid, scale=2.0)
            h_new = h_pool.tile([H, P], f32)
            nc.vector.scalar_tensor_tensor(
                h_new[:], sc[:], 0.5, s_o[:], op0=ALU.subtract, op1=ALU.mult
            )
            return h_new, c_new

        s_pools = {2: sg_pool, 0: si_pool, 1: sf_pool, 3: so_pool}

        def lstm_step(h_prev, c_prev, whT, bias_cols, rhs_x=None, prow=None,
                      wxT=None, extra_pe=None, extra_act=None):
            """One LSTM step: per-gate matmuls into per-gate PSUM banks, one
            sigmoid per gate (bias fused), then the DVE cell tail.

            extra_pe/extra_act emit off-critical-path work (decoder output
            projection) after the gate matmuls / sigmoids respectively."""
            s_tiles = {}
            banks = {}
            for g in _GORDER:
                bank = psum_g.tile([H, P], f32, tag=f"bank{g}")
                banks[g] = bank
                first = True
                if rhs_x is not None:
                    nc.tensor.matmul(
                        bank[:], wxT[prow, g * H : (g + 1) * H], rhs_x,
                        start=True, stop=False,
                    )
                    first = False
                nc.tensor.matmul(
                    bank[:], whT[:, g * H : (g + 1) * H], h_prev[:],
                    start=first, stop=True,
                )
            if extra_pe is not None:
                extra_pe()
            for g in _GORDER:
                s = s_pools[g].tile([H, P], f32)
                nc.scalar.activation(
                    s[:], banks[g][:], AF.Sigmoid, bias=bias_cols[:, g : g + 1]
                )
                s_tiles[g] = s
            if extra_act is not None:
                extra_act()
            return cell_tail(s_tiles[2], s_tiles[0], s_tiles[1], s_tiles[3],
                             c_prev)

        # ---- split-2 machinery: two independent half-batch chains.  Gates
        # are packed in (g,i) and (f,o) pair banks [H, 2*HP]; biases enter
        # via a small K=2 matmul (off the critical path), so each pair needs
        # only one sigmoid.
        HP = P // 2

        def lstm_step_half(hf, h_prev, c_prev, whT, bgi, bfo, xcols=None,
                           prow=None, wxT=None, extra_pe=None, extra_act=None):
            banks = {}
            for pair, gates, bias2 in (("gi", (2, 0), bgi), ("fo", (1, 3), bfo)):
                bank = psum_g.tile([H, 2 * HP], f32, tag=f"bank_{pair}_{hf}")
                banks[pair] = bank
                nc.tensor.matmul(bank[:], bias2[:], cst["blk2"][:],
                                 start=True, stop=False)
                for bi, g in enumerate(gates):
                    sl = bank[:, bi * HP : (bi + 1) * HP]
                    if wxT is not None:
                        nc.tensor.matmul(
                            sl, wxT[prow, g * H : (g + 1) * H], xcols,
                            start=False, stop=False,
                        )
                    nc.tensor.matmul(
                        sl, whT[:, g * H : (g + 1) * H], h_prev[:],
                        start=False, stop=(bi == 1),
                    )
            if extra_pe is not None:
                extra_pe()
            s_gi = s_pools[2].tile([H, 2 * HP], f32, tag=f"sgi{hf}")
            nc.scalar.activation(s_gi[:], banks["gi"][:], AF.Sigmoid)
            s_fo = s_pools[1].tile([H, 2 * HP], f32, tag=f"sfo{hf}")
            nc.scalar.activation(s_fo[:], banks["fo"][:], AF.Sigmoid)
            if extra_act is not None:
                extra_act()
            tp = tmp_pool.tile([H, HP], f32, tag=f"tp{hf}")
            nc.vector.scalar_tensor_tensor(
                tp[:], s_gi[:, 0:HP], 0.5, s_gi[:, HP : 2 * HP],
                op0=ALU.subtract, op1=ALU.mult,
            )
            q = tmp_pool.tile([H, HP], f32, tag=f"q{hf}")
            nc.vector.scalar_tensor_tensor(
                q[:], s_fo[:, 0:HP], 1.0, c_prev[:], op0=ALU.mult, op1=ALU.mult
            )
            c_new = c_pool.tile([H, HP], f32, tag=f"c{hf}")
            nc.vector.scalar_tensor_tensor(
                c_new[:], tp[:], 2.0, q[:], op0=ALU.mult, op1=ALU.add
            )
            sc = tmp_pool.tile([H, HP], f32, tag=f"sc{hf}")
            nc.scalar.activation(sc[:], c_new[:], AF.Sigmoid, scale=2.0)
            h_new = h_pool.tile([H, HP], f32, tag=f"h{hf}")
            nc.vector.scalar_tensor_tensor(
                h_new[:], sc[:], 0.5, s_fo[:, HP : 2 * HP],
                op0=ALU.subtract, op1=ALU.mult,
            )
            return h_new, c_new

        # ---- encoder
        if SPLIT2:
            hs, cs = [], []
            for hf in range(2):
                hh = h_pool.tile([H, HP], f32, tag=f"h{hf}")
                nc.vector.memset(hh[:], 0.0)
                cc = c_pool.tile([H, HP], f32, tag=f"c{hf}")
                nc.vector.memset(cc[:], 0.0)
                hs.append(hh)
                cs.append(cc)
            for t in range(Tn):
                par = t % 2
                prow = slice(64 * par, 64 * par + 64)
                for hf in range(2):
                    xcols = xT[prow,
                               (t // 2) * P + hf * HP : (t // 2) * P + (hf + 1) * HP]
                    hs[hf], cs[hf] = lstm_step_half(
                        hf, hs[hf], cs[hf], cst["whhT"], cst["bgi_enc"],
                        cst["bfo_enc"], xcols=xcols, prow=prow, wxT=cst["wihT"],
                    )
            for hf in range(2):
                nc.sync.dma_start(cencT_d[:, hf * HP : (hf + 1) * HP], cs[hf][:])
        else:
            h = h_pool.tile([H, P], f32)
            nc.vector.memset(h[:], 0.0)
            c = c_pool.tile([H, P], f32)
            nc.vector.memset(c[:], 0.0)
            for t in range(Tn):
                par = t % 2
                prow = slice(64 * par, 64 * par + 64)
                rhs_x = xT[prow, (t // 2) * P : (t // 2) * P + P]
                h, c = lstm_step(h, c, cst["whhT"], cst["benc"], rhs_x=rhs_x,
                                 prow=prow, wxT=cst["wihT"])
            # c after the last encoder step is the c_enc output (h is
            # h/2-scaled, c is exact)
            nc.sync.dma_start(cencT_d[:], c[:])

        # ---- decoder
        err = err_pool.tile([P, n_blk], f32)
        out_last_s = err_pool.tile([P, F], f32, tag="outlast")
        ub = None
        for k in range(Tn):
            blk, j = divmod(k, GERR)
            if j == 0:
                ub = ubuf_pool.tile([P, GERR * F], f32)
            slot = GERR - 1 - j

            if SPLIT2:
                for hf in range(2):
                    h_prev = hs[hf]
                    o_ps = psum_o.tile([HP, F], f32, tag=f"out{hf}")

                    def out_proj(o_ps=o_ps, h_prev=h_prev):
                        nc.tensor.matmul(o_ps[:], cst["ones1h"][:],
                                         cst["bout"][:], start=True, stop=False)
                        nc.tensor.matmul(o_ps[:], h_prev[:], cst["woutT"][:],
                                         start=False, stop=True)

                    def out_square(o_ps=o_ps, hf=hf):
                        dst = ub[hf * HP : (hf + 1) * HP,
                                 slot * F : (slot + 1) * F]
                        nc.scalar.activation(dst, o_ps[:], AF.Square)
                        if k == 0:
                            nc.scalar.copy(
                                out_last_s[hf * HP : (hf + 1) * HP, :], o_ps[:]
                            )

                    hs[hf], cs[hf] = lstm_step_half(
                        hf, hs[hf], cs[hf], cst["wcombT"], cst["bgi_dec"],
                        cst["bfo_dec"], extra_pe=out_proj, extra_act=out_square,
                    )
                if k == 0:
                    nc.sync.dma_start(outlast_d[:], out_last_s[:])
            else:
                h_prev = h
                o_ps = psum_o.tile([P, F], f32, tag="out0")

                def out_proj():
                    nc.tensor.matmul(o_ps[:], cst["ones1"][:], cst["bout"][:],
                                     start=True, stop=False)
                    nc.tensor.matmul(o_ps[:], h_prev[:], cst["woutT"][:],
                                     start=False, stop=True)

                def out_square():
                    nc.scalar.activation(
                        ub[:, slot * F : (slot + 1) * F], o_ps[:], AF.Square
                    )
                    if k == 0:
                        nc.scalar.copy(out_last_s[:], o_ps[:])
                        nc.sync.dma_start(outlast_d[:], out_last_s[:])

                h, c = lstm_step(h, c, cst["wcombT"], cst["bdec"],
                                 extra_pe=out_proj, extra_act=out_square)

            if j == GERR - 1:
                t_lo = Tn - 1 - k
                xr = xraw_pool.tile([P, GERR * F], f32, tag="xerr")
                nc.sync.dma_start(xr[:], x_d[:, t_lo : t_lo + GERR, :])
                xsq = big_pool.tile([P, GERR * F], f32, tag="xsq")
                nc.scalar.activation(xsq[:], xr[:], AF.Square)
                d = big_pool.tile([P, GERR * F], f32, tag="d")
                nc.vector.scalar_tensor_tensor(
                    d[:], ub[:], 1.0, xsq[:], op0=ALU.mult, op1=ALU.subtract
                )
                nc.vector.tensor_reduce(
                    err[:, blk : blk + 1], d[:], axis=AX.X, op=ALU.add,
                    apply_absolute_value=True,
                )

        err_tot = tmp_pool.tile([P, 1], f32, tag="etot")
        nc.vector.tensor_reduce(err_tot[:], err[:], axis=AX.X, op=ALU.add)
        nc.sync.dma_start(recerr_d[:], err_tot[:])

    nc.compile()
    return nc


_CACHE = {}


def _get_program(Tn=T):
    if Tn not in _CACHE:
        _CACHE[Tn] = _build_program(Tn)
    return _CACHE[Tn]


def _run(inputs, Tn=T, trace=False):
    from concourse import bass_utils

    nc = _get_program(Tn)
    consts = _prep_consts(inputs)
    x = np.asarray(inputs["ts_batch"], np.float32)
    in_maps = []
    for i in range(NCORES):
        m = dict(consts)
        m["x"] = np.ascontiguousarray(x[i * P : (i + 1) * P, :Tn, :])
        in_maps.append(m)
    res = bass_utils.run_bass_kernel_spmd(
        nc, in_maps, core_ids=list(range(NCORES)), trace=trace
    )
    c_enc = np.concatenate(
        [r["c_encT"].T for r in res.results], axis=0
    )  # [B, H]
    rec_err = np.concatenate([r["rec_err"][:, 0] for r in res.results], axis=0)
    out_last = np.concatenate([r["out_last"] for r in res.results], axis=0)
    return (c_enc, rec_err, out_last), res


def kernel(**inputs):
    (c_enc, rec_err, out_last), _ = _run(inputs)
    return c_enc, rec_err, out_last


def _timed_run(inputs, Tn=T, iters=5):
    """Measure device execution wall time with inputs pre-transferred.

    Mirrors bass2jax.run_bass_via_pjrt's multi-core path but keeps the big
    inputs resident on device across repeat calls, so the measured wall time
    is dominated by kernel execution."""
    import time

    import jax
    import numpy as np_
    from jax.sharding import Mesh, PartitionSpec
    from jax.experimental.shard_map import shard_map

    from concourse import bass2jax, mybir

    nc = _get_program(Tn)
    bass2jax.install_neuronx_cc_hook()

    consts = _prep_consts(inputs)
    x = np_.asarray(inputs["ts_batch"], np_.float32)
    in_maps = []
    for i in range(NCORES):
        m = dict(consts)
        m["x"] = np_.ascontiguousarray(x[i * P : (i + 1) * P, :Tn, :])
        in_maps.append(m)

    partition_name = (
        nc.partition_id_tensor.name if nc.partition_id_tensor else None
    )
    in_names, out_names, out_avals, zero_outs = [], [], [], []
    for alloc in nc.m.functions[0].allocations:
        if not isinstance(alloc, mybir.MemoryLocationSet):
            continue
        name = alloc.memorylocations[0].name
        if alloc.kind == "ExternalInput":
            if name != partition_name:
                in_names.append(name)
        elif alloc.kind == "ExternalOutput":
            dtype = mybir.dt.np(alloc.dtype)
            out_avals.append(
                jax.core.ShapedArray(tuple(alloc.tensor_shape), dtype)
            )
            out_names.append(name)
            zero_outs.append(np_.zeros(tuple(alloc.tensor_shape), dtype))
    n_params = len(in_names)
    n_outs = len(out_names)
    all_in_names = in_names + out_names
    if partition_name is not None:
        all_in_names.append(partition_name)

    def _body(*args):
        operands = list(args)
        if partition_name is not None:
            operands.append(bass2jax.partition_id_tensor())
        outs = bass2jax._bass_exec_p.bind(
            *operands,
            out_avals=tuple(out_avals),
            in_names=tuple(all_in_names),
            out_names=tuple(out_names),
            lowering_input_output_aliases=(),
            sim_require_finite=True,
            sim_require_nnan=True,
            nc=nc,
        )
        return tuple(outs)

    devices = jax.devices()[:NCORES]
    mesh = Mesh(np_.asarray(devices), ("core",))
    in_specs = (PartitionSpec("core"),) * (n_params + n_outs)
    out_specs = (PartitionSpec("core"),) * n_outs
    sharded = jax.jit(
        shard_map(_body, mesh=mesh, in_specs=in_specs, out_specs=out_specs,
                  check_rep=False),
        keep_unused=True,
    )
    concat_in = [
        np_.concatenate([np_.asarray(in_maps[c][nm]) for c in range(NCORES)], axis=0)
        for nm in in_names
    ]
    concat_zeros = [
        np_.zeros((NCORES * z.shape[0], *z.shape[1:]), z.dtype) for z in zero_outs
    ]
    sharding = jax.sharding.NamedSharding(mesh, PartitionSpec("core"))
    dev_in = [jax.device_put(a, sharding) for a in concat_in]
    dev_zeros = [jax.device_put(a, sharding) for a in concat_zeros]

    # warmup (includes compile)
    out = sharded(*dev_in, *dev_zeros)
    jax.block_until_ready(out)
    times = []
    for _ in range(iters):
        t0 = time.perf_counter()
        out = sharded(*dev_in, *dev_zeros)
        jax.block_until_ready(out)
        times.append(time.perf_counter() - t0)

    # amortized: queue NB async executes, block once
    NB = 16
    t0 = time.perf_counter()
    outs = [sharded(*dev_in, *dev_zeros) for _ in range(NB)]
    jax.block_until_ready(outs)
    amort = (time.perf_counter() - t0) / NB
    return min(times), times + [("amortized", amort)]
